# revision 25
# baseline (speedup 1.0000x reference)
"""Trainium2 Bass kernel for nn_Minerva_37211596652565 (retrieval_knn).

reference:
    Xn = l2norm_rows(X); Dn = l2norm_rows(D)
    a  = Xn @ Dn.T            # [N, M] cosine sims
    a  = sign(a)*|a|^3 == a^3 # odd power => plain cube
    echo = a @ r              # [N, 128]

Sharding: data-parallel over X rows across 8 NeuronCores (N_loc=1024/core),
D and r replicated. No collectives.

Host-side layout prep (pure data movement, no math):
    DT     = D.T  (contiguous [128, M])     -> mm1 stationary needs d-major
    r_perm = tile-permuted r so each SBUF partition gets a contiguous 64KB run

Per-core dataflow:
    - DT, r fully resident in SBUF; X normalized+transposed on chip (f32)
    - ss_m = sum_d D^2:  ACT Square(DT chunk)->DTsq,  PE ones-matmul -> psum col
      s = 1/sqrt(ss) per 16-tile group (ACT Sqrt + DVE reciprocal)
    - per m-tile t (128):
        PE  mm1 (f32r): aT[m,n] = DT_tile.T @ XT       (raw-D numerators)
        DVE fused custom op: a3 = (s_m * aT)^3 -> bf16 (single pass from PSUM)
        ACT copy-cast r-tile -> bf16
        PE  mm2 (bf16): echoT[k,n] += r_tile.T @ a3    (PSUM accum over t)
    - epilogue: echoT --PE transpose--> OUT[n,k]
"""

import sys

sys.path.insert(0, "/opt/trn_rl_repo")

import numpy as np

import concourse.bacc as bacc
import concourse.bass as bass
import concourse.tile as tile
from concourse import mybir
from concourse.bass_utils import run_bass_kernel_spmd
from concourse.masks import make_identity
from concourse.bass import ts

# ----------------------------------------------------------------------------
# Custom DVE op: out = (in0 * s0)^3, s0 a per-partition [P,1] scalar.
# One streaming DVE pass (3 ALU stages) replaces ACT-square + DVE-mult.
# ----------------------------------------------------------------------------
from concourse import dve_ops as dvo
from concourse.dve_spec import Spec, Src0, C0, sq, lower, _has_src1
from concourse.dve_uop import DveOpSpec


def _register_cube_op():
    name = "CUBE_SCALED_ANT"
    for op in dvo.OPS:
        if op.name == name:
            return op
    t = Src0 * C0
    spec = Spec(
        body=t * sq(t),
        reference=lambda in0, in1, s0, s1, imm2: (in0.astype(np.float32) * s0) ** 3,
    )
    row = max(dvo._SUB_OPCODE_FOR_NAME.values()) + 1
    assert row < 0x20
    dvo._SUB_OPCODE_FOR_NAME[name] = row
    shas = {}
    for ver in ("v3", "v4"):
        uops = lower(spec, ver=ver)
        shas[ver] = DveOpSpec(
            name=name, opcode=row, uops=uops, rd1_en=_has_src1(spec)
        ).sha(ver)
    op = dvo.DveOp(name, spec, subdim=False, uops_sha=shas)
    dvo.OPS.append(op)
    dvo.CUSTOM_DVE_SPECS[name] = spec
    return op


CUBE_OP = _register_cube_op()

# Problem shapes (hardcoded per contract).
N, M, d = 8192, 16384, 128
NCORES = 8
N_LOC = N // NCORES  # 1024
P = 128
NT = N_LOC // P  # 8 n-tiles per core
MT = M // P  # 128 m-tiles
GRP = 16  # m-tiles per sqrt/recip group
F32 = mybir.dt.float32
F32R = mybir.dt.float32r
BF16 = mybir.dt.bfloat16


def build_kernel(nc: bass.Bass, Xap, DTap, Rap, OUTap, tc: tile.TileContext):
    from contextlib import ExitStack

    with ExitStack() as ctx:
        consts = ctx.enter_context(tc.tile_pool(name="consts", bufs=1))
        big = ctx.enter_context(tc.tile_pool(name="big", bufs=1))
        dsqp = ctx.enter_context(tc.tile_pool(name="dsqp", bufs=2))
        rbfp = ctx.enter_context(tc.tile_pool(name="rbfp", bufs=3))
        a3p = ctx.enter_context(tc.tile_pool(name="a3p", bufs=3))
        scrapp = ctx.enter_context(tc.tile_pool(name="scrapp", bufs=2))
        xnp = ctx.enter_context(tc.tile_pool(name="xnp", bufs=2))
        outp = ctx.enter_context(tc.tile_pool(name="outp", bufs=2))

        pa = ctx.enter_context(tc.tile_pool(name="pa", bufs=2, space="PSUM"))
        pecho = ctx.enter_context(tc.tile_pool(name="pecho", bufs=1, space="PSUM"))
        pss = ctx.enter_context(tc.tile_pool(name="pss", bufs=1, space="PSUM"))
        pt = ctx.enter_context(tc.tile_pool(name="pt", bufs=1, space="PSUM"))

        ident = consts.tile([P, P], F32)
        make_identity(nc, ident)
        identb = consts.tile([P, P], BF16)
        make_identity(nc, identb)
        ones = consts.tile([P, 1], F32)
        nc.vector.memset(ones, 1.0)

        # ---- bulk loads ------------------------------------------------
        # DT staged in f32 chunks, cast on-chip to bf16 for mm1; sumsq is
        # taken from the rounded (bf16) values so cosines are self-consistent.
        DTbuf = big.tile([P, M], BF16)  # [d, m]
        Rbuf = big.tile([P, MT, d], F32)  # Rbuf[p, t, :] = r[t*128+p, :]
        Rr = Rap.rearrange("(p t) d -> p t d", t=MT)
        CH = M // 16  # 1024 cols / chunk (512 KB)
        CHT = MT // 16  # 8 m-tiles / chunk
        stagep = ctx.enter_context(tc.tile_pool(name="stagep", bufs=3))

        def emit_cast(c):
            stage = stagep.tile([P, CH], F32, tag="dstage")
            nc.sync.dma_start(out=stage, in_=DTap[:, ts(c, CH)])
            nc.vector.tensor_copy(DTbuf[:, ts(c, CH)], stage)

        # X first (it gates every mm1), contiguous 4KB descriptors via
        # permuted n-tiles: Xbuf[p, i, :] = X[p*NT + i]  (un-permuted at OUT)
        Xbuf = big.tile([P, NT, d], F32)
        Xr = Xap.rearrange("(p i) d -> p i d", i=NT)
        nc.sync.dma_start(out=Xbuf[:, :, :], in_=Xr[:, :, :])

        # ---- X transpose into XT [d, n_loc], RAW (un-normalized) --------
        # X norm is deferred: echo rows get scaled by sx^3 in the epilogue
        # (cube is homogeneous degree 3), keeping X-prep off the critical path.
        XT = consts.tile([P, N_LOC], BF16)  # [d, n]
        for i in range(NT):
            xb = xnp.tile([P, d], BF16, tag="xb")
            nc.vector.tensor_copy(xb, Xbuf[:, i, :])
            ptb = pt.tile([P, P], BF16, tag="pt")
            nc.tensor.transpose(ptb, xb, identb)
            nc.scalar.activation(
                out=XT[:, ts(i, P)],
                in_=ptb,
                func=mybir.ActivationFunctionType.Copy,
            )

        # early D chunks so mm1/ss can start, then r bulk
        for c in range(4):
            emit_cast(c)
        for c in range(16):
            nc.sync.dma_start(
                out=Rbuf[:, ts(c, CHT), :], in_=Rr[:, ts(c, CHT), :]
            )

        # ---- X norms (needed only by the epilogue) ----------------------
        ssx = consts.tile([P, NT], F32)
        sx3 = consts.tile([P, NT], F32)
        xsq = consts.tile([P, NT, d], F32)
        nc.vector.tensor_mul(
            xsq.rearrange("p a b -> p (a b)"),
            Xbuf.rearrange("p a b -> p (a b)"),
            Xbuf.rearrange("p a b -> p (a b)"),
        )
        nc.vector.tensor_reduce(
            ssx, xsq, axis=mybir.AxisListType.X, op=mybir.AluOpType.add
        )
        nc.scalar.activation(out=sx3, in_=ssx, func=mybir.ActivationFunctionType.Sqrt)
        nc.vector.reciprocal(out=sx3, in_=sx3)
        nc.vector.tensor_mul(ssx, sx3, sx3)  # reuse ssx as sx^2
        nc.vector.tensor_mul(sx3, ssx, sx3)  # sx^3

        # ---- D row norms: ss_m = sum_d DT[:,m]^2 via Square + ones-matmul.
        # Emitted group-by-group, interleaved with the main loop so the
        # pipeline ramps immediately instead of waiting for all of D.
        ss_ps = pss.tile([P, MT], F32)  # psum, col t = ss for m-tile t
        sd = consts.tile([P, MT], F32)

        def emit_ss_group(g):
            # sumsq + rsqrt for m-tiles [g*GRP, (g+1)*GRP)
            for c in range(g * GRP // 4, (g + 1) * GRP // 4):  # 512-col chunks
                dsq = dsqp.tile([P, 512], F32, tag="dsq")
                nc.scalar.activation(
                    out=dsq,
                    in_=DTbuf[:, ts(c, 512)],
                    func=mybir.ActivationFunctionType.Square,
                )
                for k in range(4):
                    t = 4 * c + k
                    nc.tensor.matmul(
                        ss_ps[:, t : t + 1],
                        lhsT=dsq[:, ts(k, P)],
                        rhs=ones,
                        start=True,
                        stop=True,
                    )
            nc.scalar.activation(
                out=sd[:, ts(g, GRP)],
                in_=ss_ps[:, ts(g, GRP)],
                func=mybir.ActivationFunctionType.Sqrt,
            )
            nc.vector.reciprocal(out=sd[:, ts(g, GRP)], in_=sd[:, ts(g, GRP)])

        emit_ss_group(0)

        # ---- main loop over m-tiles ------------------------------------
        echoT = pecho.tile([P, N_LOC], F32)  # [k, n] psum accumulator
        for t in range(MT):
            aT = pa.tile([P, N_LOC], F32, tag="aT")
            lhsT1 = DTbuf[:, ts(t, P)]
            for c in range(N_LOC // 512):
                nc.tensor.matmul(
                    aT[:, ts(c, 512)],
                    lhsT=lhsT1,
                    rhs=XT[:, ts(c, 512)],
                    start=True,
                    stop=True,
                )

            # fused cube with per-partition norm scale: a3 = (s_m * aT)^3
            a3 = a3p.tile([P, N_LOC], BF16, tag="a3")
            nc.vector._custom_dve(CUBE_OP, out=a3, in0=aT, s0=sd[:, t : t + 1])

            # r tile -> bf16
            rbf = rbfp.tile([P, P], BF16, tag="rbf")
            nc.scalar.activation(
                out=rbf, in_=Rbuf[:, t, :], func=mybir.ActivationFunctionType.Copy
            )

            # mm2: echoT[k, n] += r_tile.T @ a3
            for c in range(N_LOC // 512):
                nc.tensor.matmul(
                    echoT[:, ts(c, 512)],
                    lhsT=rbf,
                    rhs=a3[:, ts(c, 512)],
                    start=(t == 0),
                    stop=(t == MT - 1),
                )

            # prefetch work emitted at tile tails so it sits AFTER this
            # tile's cube in each engine's stream (no head-of-line blocks)
            if t % 8 == 7 and t // 8 + 4 < 16:
                emit_cast(t // 8 + 4)
            if t % GRP == 8 and t // GRP + 1 < MT // GRP:
                emit_ss_group(t // GRP + 1)

        # ---- epilogue: transpose echoT -> OUT [n, k] --------------------
        echoS = consts.tile([P, N_LOC], F32)
        nc.scalar.activation(
            out=echoS, in_=echoT, func=mybir.ActivationFunctionType.Copy
        )
        OUTr = OUTap.rearrange("(p i) d -> p i d", i=NT)
        for i in range(NT):
            ptile = pt.tile([P, P], F32, tag="pt")
            nc.tensor.transpose(ptile, echoS[:, ts(i, P)], ident)
            otile = outp.tile([P, P], F32, tag="otile")
            nc.vector.tensor_scalar_mul(otile, ptile, sx3[:, i : i + 1])
            nc.sync.dma_start(out=OUTr[:, i, :], in_=otile)


_COMPILED = None


def _get_compiled():
    global _COMPILED
    if _COMPILED is None:
        nc = bacc.Bacc(
            "TRN2",
            target_bir_lowering=False,
            debug=False,
            num_devices=1,
        )
        Xap = nc.dram_tensor("X", [N_LOC, d], F32, kind="ExternalInput").ap()
        DTap = nc.dram_tensor("DT", [d, M], F32, kind="ExternalInput").ap()
        Rap = nc.dram_tensor("RP", [M, d], F32, kind="ExternalInput").ap()
        OUTap = nc.dram_tensor("OUT", [N_LOC, d], F32, kind="ExternalOutput").ap()
        with tile.TileContext(nc) as tc:
            build_kernel(nc, Xap, DTap, Rap, OUTap, tc)
        nc.compile()
        _COMPILED = nc
    return _COMPILED


def kernel(X, D, r, _trace=False, _trace_kwargs=None):
    X = np.ascontiguousarray(np.asarray(X), dtype=np.float32)
    D = np.ascontiguousarray(np.asarray(D), dtype=np.float32)
    r = np.ascontiguousarray(np.asarray(r), dtype=np.float32)
    assert X.shape == (N, d) and D.shape == (M, d) and r.shape == (M, d)

    # host-side layout prep (no math): transpose D, tile-permute r
    DT = np.ascontiguousarray(D.T)  # [128, M]
    r_perm = np.ascontiguousarray(
        r.reshape(MT, P, d).transpose(1, 0, 2).reshape(M, d)
    )  # r_perm[p*128+t] = r[t*128+p]

    nc = _get_compiled()
    in_maps = [
        {
            "X": np.ascontiguousarray(X[c * N_LOC : (c + 1) * N_LOC]),
            "DT": DT,
            "RP": r_perm,
        }
        for c in range(NCORES)
    ]
    res = run_bass_kernel_spmd(
        nc,
        in_maps,
        core_ids=list(range(NCORES)),
        trace=_trace,
        **(_trace_kwargs or {}),
    )
    out = np.concatenate([res.results[c]["OUT"] for c in range(NCORES)], axis=0)
    if _trace:
        kernel._last_results = res
    return out


# revision 29
# speedup vs baseline: 1.4061x; 1.4061x over previous
"""Trainium2 Bass kernel for nn_Minerva_37211596652565 (retrieval_knn).

reference:
    Xn = l2norm_rows(X); Dn = l2norm_rows(D)
    a  = Xn @ Dn.T            # [N, M] cosine sims
    a  = sign(a)*|a|^3 == a^3 # odd power => plain cube
    echo = a @ r              # [N, 128]

Sharding: data-parallel over X rows across 8 NeuronCores (N_loc=1024/core),
D and r replicated. No collectives.

Host-side layout prep (pure data movement, no math):
    DT     = D.T  (contiguous [128, M])     -> mm1 stationary needs d-major
    r_perm = tile-permuted r so each SBUF partition gets a contiguous 64KB run

Per-core dataflow:
    - DT, r fully resident in SBUF; X normalized+transposed on chip (f32)
    - ss_m = sum_d D^2:  ACT Square(DT chunk)->DTsq,  PE ones-matmul -> psum col
      s = 1/sqrt(ss) per 16-tile group (ACT Sqrt + DVE reciprocal)
    - per m-tile t (128):
        PE  mm1 (f32r): aT[m,n] = DT_tile.T @ XT       (raw-D numerators)
        DVE fused custom op: a3 = (s_m * aT)^3 -> bf16 (single pass from PSUM)
        ACT copy-cast r-tile -> bf16
        PE  mm2 (bf16): echoT[k,n] += r_tile.T @ a3    (PSUM accum over t)
    - epilogue: echoT --PE transpose--> OUT[n,k]
"""

import sys

sys.path.insert(0, "/opt/trn_rl_repo")

import numpy as np

import concourse.bacc as bacc
import concourse.bass as bass
import concourse.tile as tile
from concourse import mybir
from concourse.bass_utils import run_bass_kernel_spmd
from concourse.masks import make_identity
from concourse.bass import ts

# ----------------------------------------------------------------------------
# Custom DVE op: out = (in0 * s0)^3, s0 a per-partition [P,1] scalar.
# One streaming DVE pass (3 ALU stages) replaces ACT-square + DVE-mult.
# ----------------------------------------------------------------------------
from concourse import dve_ops as dvo
from concourse.dve_spec import Spec, Src0, C0, sq, lower, _has_src1
from concourse.dve_uop import DveOpSpec


def _register_cube_op():
    name = "CUBE_SCALED_ANT"
    for op in dvo.OPS:
        if op.name == name:
            return op
    t = Src0 * C0
    spec = Spec(
        body=t * sq(t),
        reference=lambda in0, in1, s0, s1, imm2: (in0.astype(np.float32) * s0) ** 3,
    )
    row = max(dvo._SUB_OPCODE_FOR_NAME.values()) + 1
    assert row < 0x20
    dvo._SUB_OPCODE_FOR_NAME[name] = row
    shas = {}
    for ver in ("v3", "v4"):
        uops = lower(spec, ver=ver)
        shas[ver] = DveOpSpec(
            name=name, opcode=row, uops=uops, rd1_en=_has_src1(spec)
        ).sha(ver)
    op = dvo.DveOp(name, spec, subdim=False, uops_sha=shas)
    dvo.OPS.append(op)
    dvo.CUSTOM_DVE_SPECS[name] = spec
    return op


CUBE_OP = _register_cube_op()

# Problem shapes (hardcoded per contract).
N, M, d = 8192, 16384, 128
NCORES = 8
N_LOC = N // NCORES  # 1024
P = 128
NT = N_LOC // P  # 8 n-tiles per core
MT = M // P  # 128 m-tiles
GRP = 16  # m-tiles per sqrt/recip group
F32 = mybir.dt.float32
F32R = mybir.dt.float32r
BF16 = mybir.dt.bfloat16


def build_kernel(nc: bass.Bass, Xap, DTap, Rap, OUTap, tc: tile.TileContext):
    from contextlib import ExitStack

    with ExitStack() as ctx:
        consts = ctx.enter_context(tc.tile_pool(name="consts", bufs=1))
        big = ctx.enter_context(tc.tile_pool(name="big", bufs=1))
        dsqp = ctx.enter_context(tc.tile_pool(name="dsqp", bufs=2))
        rbfp = ctx.enter_context(tc.tile_pool(name="rbfp", bufs=3))
        a3p = ctx.enter_context(tc.tile_pool(name="a3p", bufs=3))
        scrapp = ctx.enter_context(tc.tile_pool(name="scrapp", bufs=2))
        xnp = ctx.enter_context(tc.tile_pool(name="xnp", bufs=2))
        outp = ctx.enter_context(tc.tile_pool(name="outp", bufs=2))

        pa = ctx.enter_context(tc.tile_pool(name="pa", bufs=2, space="PSUM"))
        pecho = ctx.enter_context(tc.tile_pool(name="pecho", bufs=1, space="PSUM"))
        pss = ctx.enter_context(tc.tile_pool(name="pss", bufs=1, space="PSUM"))
        pt = ctx.enter_context(tc.tile_pool(name="pt", bufs=1, space="PSUM"))

        identb = consts.tile([P, P], BF16)
        make_identity(nc, identb)
        ones = consts.tile([P, 1], BF16)
        nc.vector.memset(ones, 1.0)

        # ---- bulk loads ------------------------------------------------
        # DT staged in f32 chunks, cast on-chip to bf16 for mm1; sumsq is
        # taken from the rounded (bf16) values so cosines are self-consistent.
        DTbuf = big.tile([P, M], BF16)  # [d, m]
        Rbuf = big.tile([P, MT, d], F32)  # Rbuf[p, t, :] = r[t*128+p, :]
        Rr = Rap.rearrange("(p t) d -> p t d", t=MT)
        CH = M // 16  # 1024 cols / chunk (512 KB)
        CHT = MT // 16  # 8 m-tiles / chunk
        stagep = ctx.enter_context(tc.tile_pool(name="stagep", bufs=3))

        def emit_cast(c):
            stage = stagep.tile([P, CH], F32, tag="dstage")
            nc.sync.dma_start(out=stage, in_=DTap[:, ts(c, CH)])
            nc.vector.tensor_copy(DTbuf[:, ts(c, CH)], stage)

        # X first (it gates every mm1), contiguous 4KB descriptors via
        # permuted n-tiles: Xbuf[p, i, :] = X[p*NT + i]  (un-permuted at OUT)
        Xbuf = big.tile([P, NT, d], F32)
        Xr = Xap.rearrange("(p i) d -> p i d", i=NT)
        nc.sync.dma_start(out=Xbuf[:, :, :], in_=Xr[:, :, :])

        # ---- X transpose into XT [d, n_loc], RAW (un-normalized) --------
        # X norm is deferred: echo rows get scaled by sx^3 in the epilogue
        # (cube is homogeneous degree 3), keeping X-prep off the critical path.
        XT = consts.tile([P, N_LOC], BF16)  # [d, n]
        for i in range(NT):
            xb = xnp.tile([P, d], BF16, tag="xb")
            nc.vector.tensor_copy(xb, Xbuf[:, i, :])
            ptb = pt.tile([P, P], BF16, tag="pt")
            nc.tensor.transpose(ptb, xb, identb)
            nc.scalar.activation(
                out=XT[:, ts(i, P)],
                in_=ptb,
                func=mybir.ActivationFunctionType.Copy,
            )

        # early D chunks so mm1/ss can start, then r bulk
        for c in range(4):
            emit_cast(c)
        for c in range(16):
            nc.sync.dma_start(
                out=Rbuf[:, ts(c, CHT), :], in_=Rr[:, ts(c, CHT), :]
            )

        # ---- X norms (needed only by the epilogue) ----------------------
        ssx = consts.tile([P, NT], F32)
        sx3 = consts.tile([P, NT], F32)
        xsq = consts.tile([P, NT, d], F32)
        nc.vector.tensor_mul(
            xsq.rearrange("p a b -> p (a b)"),
            Xbuf.rearrange("p a b -> p (a b)"),
            Xbuf.rearrange("p a b -> p (a b)"),
        )
        nc.vector.tensor_reduce(
            ssx, xsq, axis=mybir.AxisListType.X, op=mybir.AluOpType.add
        )
        nc.scalar.activation(out=sx3, in_=ssx, func=mybir.ActivationFunctionType.Sqrt)
        nc.vector.reciprocal(out=sx3, in_=sx3)
        nc.vector.tensor_mul(ssx, sx3, sx3)  # reuse ssx as sx^2
        nc.vector.tensor_mul(sx3, ssx, sx3)  # sx^3

        # ---- D row norms: ss_m = sum_d DT[:,m]^2 via Square + ones-matmul.
        # Emitted group-by-group, interleaved with the main loop so the
        # pipeline ramps immediately instead of waiting for all of D.
        ss_ps = pss.tile([P, MT], F32)  # psum, col t = ss for m-tile t
        sd = consts.tile([P, MT], F32)

        def emit_ss_group(g):
            # sumsq + rsqrt for m-tiles [g*GRP, (g+1)*GRP)
            for c in range(g * GRP // 4, (g + 1) * GRP // 4):  # 512-col chunks
                dsq = dsqp.tile([P, 512], BF16, tag="dsq")
                nc.scalar.activation(
                    out=dsq,
                    in_=DTbuf[:, ts(c, 512)],
                    func=mybir.ActivationFunctionType.Square,
                )
                for k in range(4):
                    t = 4 * c + k
                    nc.tensor.matmul(
                        ss_ps[:, t : t + 1],
                        lhsT=dsq[:, ts(k, P)],
                        rhs=ones,
                        start=True,
                        stop=True,
                    )
            nc.scalar.activation(
                out=sd[:, ts(g, GRP)],
                in_=ss_ps[:, ts(g, GRP)],
                func=mybir.ActivationFunctionType.Sqrt,
            )
            nc.vector.reciprocal(out=sd[:, ts(g, GRP)], in_=sd[:, ts(g, GRP)])

        emit_ss_group(0)

        # ---- main loop over m-tiles ------------------------------------
        echoT = pecho.tile([P, N_LOC], F32)  # [k, n] psum accumulator
        for t in range(MT):
            aT = pa.tile([P, N_LOC], F32, tag="aT")
            lhsT1 = DTbuf[:, ts(t, P)]
            for c in range(N_LOC // 512):
                nc.tensor.matmul(
                    aT[:, ts(c, 512)],
                    lhsT=lhsT1,
                    rhs=XT[:, ts(c, 512)],
                    start=True,
                    stop=True,
                )

            # fused cube with per-partition norm scale: a3 = (s_m * aT)^3
            a3 = a3p.tile([P, N_LOC], BF16, tag="a3")
            nc.vector._custom_dve(CUBE_OP, out=a3, in0=aT, s0=sd[:, t : t + 1])

            # r tile -> bf16
            rbf = rbfp.tile([P, P], BF16, tag="rbf")
            nc.scalar.activation(
                out=rbf, in_=Rbuf[:, t, :], func=mybir.ActivationFunctionType.Copy
            )

            # mm2: echoT[k, n] += r_tile.T @ a3
            for c in range(N_LOC // 512):
                nc.tensor.matmul(
                    echoT[:, ts(c, 512)],
                    lhsT=rbf,
                    rhs=a3[:, ts(c, 512)],
                    start=(t == 0),
                    stop=(t == MT - 1),
                )

            # prefetch work emitted at tile tails so it sits AFTER this
            # tile's cube in each engine's stream (no head-of-line blocks)
            if t % 8 == 7 and t // 8 + 4 < 16:
                emit_cast(t // 8 + 4)
            if t % GRP == 8 and t // GRP + 1 < MT // GRP:
                emit_ss_group(t // GRP + 1)

        # ---- epilogue: transpose echoT -> OUT [n, k] --------------------
        echoS = consts.tile([P, N_LOC], BF16)
        nc.scalar.activation(
            out=echoS, in_=echoT, func=mybir.ActivationFunctionType.Copy
        )
        OUTr = OUTap.rearrange("(p i) d -> p i d", i=NT)
        for i in range(NT):
            ptile = pt.tile([P, P], BF16, tag="pt")
            nc.tensor.transpose(ptile, echoS[:, ts(i, P)], identb)
            otile = outp.tile([P, P], F32, tag="otile")
            nc.vector.tensor_scalar_mul(otile, ptile, sx3[:, i : i + 1])
            nc.sync.dma_start(out=OUTr[:, i, :], in_=otile)


_COMPILED = None


def _get_compiled():
    global _COMPILED
    if _COMPILED is None:
        nc = bacc.Bacc(
            "TRN2",
            target_bir_lowering=False,
            debug=False,
            num_devices=1,
        )
        Xap = nc.dram_tensor("X", [N_LOC, d], F32, kind="ExternalInput").ap()
        DTap = nc.dram_tensor("DT", [d, M], F32, kind="ExternalInput").ap()
        Rap = nc.dram_tensor("RP", [M, d], F32, kind="ExternalInput").ap()
        OUTap = nc.dram_tensor("OUT", [N_LOC, d], F32, kind="ExternalOutput").ap()
        with tile.TileContext(nc) as tc:
            build_kernel(nc, Xap, DTap, Rap, OUTap, tc)
        nc.compile()
        _COMPILED = nc
    return _COMPILED


def kernel(X, D, r, _trace=False, _trace_kwargs=None):
    X = np.ascontiguousarray(np.asarray(X), dtype=np.float32)
    D = np.ascontiguousarray(np.asarray(D), dtype=np.float32)
    r = np.ascontiguousarray(np.asarray(r), dtype=np.float32)
    assert X.shape == (N, d) and D.shape == (M, d) and r.shape == (M, d)

    # host-side layout prep (no math): transpose D, tile-permute r
    DT = np.ascontiguousarray(D.T)  # [128, M]
    r_perm = np.ascontiguousarray(
        r.reshape(MT, P, d).transpose(1, 0, 2).reshape(M, d)
    )  # r_perm[p*128+t] = r[t*128+p]

    nc = _get_compiled()
    in_maps = [
        {
            "X": np.ascontiguousarray(X[c * N_LOC : (c + 1) * N_LOC]),
            "DT": DT,
            "RP": r_perm,
        }
        for c in range(NCORES)
    ]
    res = run_bass_kernel_spmd(
        nc,
        in_maps,
        core_ids=list(range(NCORES)),
        trace=_trace,
        **(_trace_kwargs or {}),
    )
    out = np.concatenate([res.results[c]["OUT"] for c in range(NCORES)], axis=0)
    if _trace:
        kernel._last_results = res
    return out


# revision 35
# speedup vs baseline: 1.4252x; 1.0136x over previous
"""Trainium2 Bass kernel for nn_Minerva_37211596652565 (retrieval_knn).

reference:
    Xn = l2norm_rows(X); Dn = l2norm_rows(D)
    a  = Xn @ Dn.T            # [N, M] cosine sims
    a  = sign(a)*|a|^3 == a^3 # odd power => plain cube
    echo = a @ r              # [N, 128]

Sharding: data-parallel over X rows across 8 NeuronCores (N_loc=1024/core),
D and r replicated. No collectives.

Host-side layout prep (pure data movement, no math):
    DT     = D.T  (contiguous [128, M])     -> mm1 stationary needs d-major
    r_perm = tile-permuted r so each SBUF partition gets a contiguous 64KB run

Per-core dataflow:
    - DT, r fully resident in SBUF; X normalized+transposed on chip (f32)
    - ss_m = sum_d D^2:  ACT Square(DT chunk)->DTsq,  PE ones-matmul -> psum col
      s = 1/sqrt(ss) per 16-tile group (ACT Sqrt + DVE reciprocal)
    - per m-tile t (128):
        PE  mm1 (f32r): aT[m,n] = DT_tile.T @ XT       (raw-D numerators)
        DVE fused custom op: a3 = (s_m * aT)^3 -> bf16 (single pass from PSUM)
        ACT copy-cast r-tile -> bf16
        PE  mm2 (bf16): echoT[k,n] += r_tile.T @ a3    (PSUM accum over t)
    - epilogue: echoT --PE transpose--> OUT[n,k]
"""

import sys

sys.path.insert(0, "/opt/trn_rl_repo")

import numpy as np

import concourse.bacc as bacc
import concourse.bass as bass
import concourse.tile as tile
from concourse import mybir
from concourse.bass_utils import run_bass_kernel_spmd
from concourse.masks import make_identity
from concourse.bass import ts

# ----------------------------------------------------------------------------
# Custom DVE op: out = (in0 * s0)^3, s0 a per-partition [P,1] scalar.
# One streaming DVE pass (3 ALU stages) replaces ACT-square + DVE-mult.
# ----------------------------------------------------------------------------
from concourse import dve_ops as dvo
from concourse.dve_spec import Spec, Src0, C0, sq, lower, _has_src1
from concourse.dve_uop import DveOpSpec


def _register_cube_op():
    name = "CUBE_SCALED_ANT"
    for op in dvo.OPS:
        if op.name == name:
            return op
    t = Src0 * C0
    spec = Spec(
        body=t * sq(t),
        reference=lambda in0, in1, s0, s1, imm2: (in0.astype(np.float32) * s0) ** 3,
    )
    row = max(dvo._SUB_OPCODE_FOR_NAME.values()) + 1
    assert row < 0x20
    dvo._SUB_OPCODE_FOR_NAME[name] = row
    shas = {}
    for ver in ("v3", "v4"):
        uops = lower(spec, ver=ver)
        shas[ver] = DveOpSpec(
            name=name, opcode=row, uops=uops, rd1_en=_has_src1(spec)
        ).sha(ver)
    op = dvo.DveOp(name, spec, subdim=False, uops_sha=shas)
    dvo.OPS.append(op)
    dvo.CUSTOM_DVE_SPECS[name] = spec
    return op


CUBE_OP = _register_cube_op()

# Problem shapes (hardcoded per contract).
N, M, d = 8192, 16384, 128
NCORES = 8
N_LOC = N // NCORES  # 1024
P = 128
NT = N_LOC // P  # 8 n-tiles per core
MT = M // P  # 128 m-tiles
GRP = 16  # m-tiles per sqrt/recip group
F32 = mybir.dt.float32
F32R = mybir.dt.float32r
BF16 = mybir.dt.bfloat16


def build_kernel(nc: bass.Bass, Xap, DTap, Rap, OUTap, tc: tile.TileContext):
    from contextlib import ExitStack

    with ExitStack() as ctx:
        consts = ctx.enter_context(tc.tile_pool(name="consts", bufs=1))
        big = ctx.enter_context(tc.tile_pool(name="big", bufs=1))
        dsqp = ctx.enter_context(tc.tile_pool(name="dsqp", bufs=2))
        rbfp = ctx.enter_context(tc.tile_pool(name="rbfp", bufs=3))
        a3p = ctx.enter_context(tc.tile_pool(name="a3p", bufs=3))
        scrapp = ctx.enter_context(tc.tile_pool(name="scrapp", bufs=2))
        xnp = ctx.enter_context(tc.tile_pool(name="xnp", bufs=2))
        outp = ctx.enter_context(tc.tile_pool(name="outp", bufs=2))

        pa = ctx.enter_context(tc.tile_pool(name="pa", bufs=2, space="PSUM"))
        pecho = ctx.enter_context(tc.tile_pool(name="pecho", bufs=1, space="PSUM"))
        pss = ctx.enter_context(tc.tile_pool(name="pss", bufs=1, space="PSUM"))
        pt = ctx.enter_context(tc.tile_pool(name="pt", bufs=1, space="PSUM"))

        identb = consts.tile([P, P], BF16)
        make_identity(nc, identb)
        ones = consts.tile([P, 1], BF16)
        nc.vector.memset(ones, 1.0)

        # ---- bulk loads ------------------------------------------------
        # DT staged in f32 chunks, cast on-chip to bf16 for mm1; sumsq is
        # taken from the rounded (bf16) values so cosines are self-consistent.
        DTbuf = big.tile([P, M], BF16)  # [d, m]
        Rbuf = big.tile([P, MT, d], F32)  # Rbuf[p, t, :] = r[t*128+p, :]
        Rr = Rap.rearrange("(p t) d -> p t d", t=MT)
        CH = M // 16  # 1024 cols / chunk (512 KB)
        CHT = MT // 16  # 8 m-tiles / chunk
        stagep = ctx.enter_context(tc.tile_pool(name="stagep", bufs=3))

        def emit_cast(c):
            stage = stagep.tile([P, CH], F32, tag="dstage")
            nc.sync.dma_start(out=stage, in_=DTap[:, ts(c, CH)])
            nc.scalar.activation(
                out=DTbuf[:, ts(c, CH)],
                in_=stage,
                func=mybir.ActivationFunctionType.Copy,
            )

        # X first (it gates every mm1), contiguous 4KB descriptors via
        # permuted n-tiles: Xbuf[p, i, :] = X[p*NT + i]  (un-permuted at OUT)
        Xbuf = big.tile([P, NT, d], F32)
        Xr = Xap.rearrange("(p i) d -> p i d", i=NT)
        nc.sync.dma_start(out=Xbuf[:, :, :], in_=Xr[:, :, :])

        # ---- X transpose into XT [d, n_loc], RAW (un-normalized) --------
        # X norm is deferred: echo rows get scaled by sx^3 in the epilogue
        # (cube is homogeneous degree 3), keeping X-prep off the critical path.
        XT = consts.tile([P, N_LOC], BF16)  # [d, n]
        for i in range(NT):
            xb = xnp.tile([P, d], BF16, tag="xb")
            nc.vector.tensor_copy(xb, Xbuf[:, i, :])
            ptb = pt.tile([P, P], BF16, tag="pt")
            nc.tensor.transpose(ptb, xb, identb)
            nc.scalar.activation(
                out=XT[:, ts(i, P)],
                in_=ptb,
                func=mybir.ActivationFunctionType.Copy,
            )

        # early D chunks so mm1/ss can start, then r bulk
        for c in range(4):
            emit_cast(c)
        for c in range(16):
            nc.sync.dma_start(
                out=Rbuf[:, ts(c, CHT), :], in_=Rr[:, ts(c, CHT), :]
            )

        # ---- X norms (consumed only by the epilogue; computed in the
        # DVE's idle ramp window) -----------------------------------------
        ssx = consts.tile([P, NT], F32)
        sx3 = consts.tile([P, NT], F32)
        xsq = consts.tile([P, NT, d], F32)
        nc.vector.tensor_mul(
            xsq.rearrange("p a b -> p (a b)"),
            Xbuf.rearrange("p a b -> p (a b)"),
            Xbuf.rearrange("p a b -> p (a b)"),
        )
        nc.vector.tensor_reduce(
            ssx, xsq, axis=mybir.AxisListType.X, op=mybir.AluOpType.add
        )
        nc.scalar.activation(out=sx3, in_=ssx, func=mybir.ActivationFunctionType.Sqrt)
        nc.vector.reciprocal(out=sx3, in_=sx3)
        nc.vector.tensor_mul(ssx, sx3, sx3)  # reuse ssx as sx^2
        nc.vector.tensor_mul(sx3, ssx, sx3)  # sx^3

        # ---- D row norms: ss_m = sum_d DT[:,m]^2 via Square + ones-matmul.
        # Emitted group-by-group, interleaved with the main loop so the
        # pipeline ramps immediately instead of waiting for all of D.
        ss_ps = pss.tile([P, MT], F32)  # psum, col t = ss for m-tile t
        sd = consts.tile([P, MT], F32)

        def emit_ss_group(g):
            # sumsq + rsqrt for m-tiles [g*GRP, (g+1)*GRP)
            for c in range(g * GRP // 8, (g + 1) * GRP // 8):  # 1024-col chunks
                dsq = dsqp.tile([P, 1024], BF16, tag="dsq")
                nc.scalar.activation(
                    out=dsq,
                    in_=DTbuf[:, ts(c, 1024)],
                    func=mybir.ActivationFunctionType.Square,
                )
                for k in range(8):
                    t = 8 * c + k
                    nc.tensor.matmul(
                        ss_ps[:, t : t + 1],
                        lhsT=dsq[:, ts(k, P)],
                        rhs=ones,
                        start=True,
                        stop=True,
                    )
            nc.scalar.activation(
                out=sd[:, ts(g, GRP)],
                in_=ss_ps[:, ts(g, GRP)],
                func=mybir.ActivationFunctionType.Sqrt,
            )
            nc.vector.reciprocal(out=sd[:, ts(g, GRP)], in_=sd[:, ts(g, GRP)])

        emit_ss_group(0)

        # ---- main loop over m-tiles ------------------------------------
        echoT = pecho.tile([P, N_LOC], F32)  # [k, n] psum accumulator
        for t in range(MT):
            aT = pa.tile([P, N_LOC], F32, tag="aT")
            lhsT1 = DTbuf[:, ts(t, P)]
            for c in range(N_LOC // 512):
                nc.tensor.matmul(
                    aT[:, ts(c, 512)],
                    lhsT=lhsT1,
                    rhs=XT[:, ts(c, 512)],
                    start=True,
                    stop=True,
                )

            # fused cube with per-partition norm scale: a3 = (s_m * aT)^3
            a3 = a3p.tile([P, N_LOC], BF16, tag="a3")
            nc.vector._custom_dve(CUBE_OP, out=a3, in0=aT, s0=sd[:, t : t + 1])

            # r tile -> bf16
            rbf = rbfp.tile([P, P], BF16, tag="rbf")
            nc.scalar.activation(
                out=rbf, in_=Rbuf[:, t, :], func=mybir.ActivationFunctionType.Copy
            )

            # mm2: echoT[k, n] += r_tile.T @ a3
            for c in range(N_LOC // 512):
                nc.tensor.matmul(
                    echoT[:, ts(c, 512)],
                    lhsT=rbf,
                    rhs=a3[:, ts(c, 512)],
                    start=(t == 0),
                    stop=(t == MT - 1),
                )

            # prefetch work emitted at tile tails so it sits AFTER this
            # tile's cube in each engine's stream (no head-of-line blocks)
            if t % 8 == 7 and t // 8 + 4 < 16:
                emit_cast(t // 8 + 4)
            if t % GRP == 8 and t // GRP + 1 < MT // GRP:
                emit_ss_group(t // GRP + 1)

        # ---- epilogue: transpose echoT -> OUT [n, k] --------------------
        echoS = consts.tile([P, N_LOC], BF16)
        nc.scalar.activation(
            out=echoS, in_=echoT, func=mybir.ActivationFunctionType.Copy
        )
        OUTr = OUTap.rearrange("(p i) d -> p i d", i=NT)
        for i in range(NT):
            ptile = pt.tile([P, P], BF16, tag="pt")
            nc.tensor.transpose(ptile, echoS[:, ts(i, P)], identb)
            otile = outp.tile([P, P], F32, tag="otile")
            nc.vector.tensor_scalar_mul(otile, ptile, sx3[:, i : i + 1])
            nc.sync.dma_start(out=OUTr[:, i, :], in_=otile)


_COMPILED = None


def _get_compiled():
    global _COMPILED
    if _COMPILED is None:
        nc = bacc.Bacc(
            "TRN2",
            target_bir_lowering=False,
            debug=False,
            num_devices=1,
        )
        Xap = nc.dram_tensor("X", [N_LOC, d], F32, kind="ExternalInput").ap()
        DTap = nc.dram_tensor("DT", [d, M], F32, kind="ExternalInput").ap()
        Rap = nc.dram_tensor("RP", [M, d], F32, kind="ExternalInput").ap()
        OUTap = nc.dram_tensor("OUT", [N_LOC, d], F32, kind="ExternalOutput").ap()
        with tile.TileContext(nc) as tc:
            build_kernel(nc, Xap, DTap, Rap, OUTap, tc)
        nc.compile()
        _COMPILED = nc
    return _COMPILED


def kernel(X, D, r, _trace=False, _trace_kwargs=None):
    X = np.ascontiguousarray(np.asarray(X), dtype=np.float32)
    D = np.ascontiguousarray(np.asarray(D), dtype=np.float32)
    r = np.ascontiguousarray(np.asarray(r), dtype=np.float32)
    assert X.shape == (N, d) and D.shape == (M, d) and r.shape == (M, d)

    # host-side layout prep (no math): transpose D, tile-permute r
    DT = np.ascontiguousarray(D.T)  # [128, M]
    r_perm = np.ascontiguousarray(
        r.reshape(MT, P, d).transpose(1, 0, 2).reshape(M, d)
    )  # r_perm[p*128+t] = r[t*128+p]

    nc = _get_compiled()
    in_maps = [
        {
            "X": np.ascontiguousarray(X[c * N_LOC : (c + 1) * N_LOC]),
            "DT": DT,
            "RP": r_perm,
        }
        for c in range(NCORES)
    ]
    res = run_bass_kernel_spmd(
        nc,
        in_maps,
        core_ids=list(range(NCORES)),
        trace=_trace,
        **(_trace_kwargs or {}),
    )
    out = np.concatenate([res.results[c]["OUT"] for c in range(NCORES)], axis=0)
    if _trace:
        kernel._last_results = res
    return out


# revision 39
# speedup vs baseline: 1.4523x; 1.0190x over previous
"""Trainium2 Bass kernel for nn_Minerva_37211596652565 (retrieval_knn).

reference:
    Xn = l2norm_rows(X); Dn = l2norm_rows(D)
    a  = Xn @ Dn.T            # [N, M] cosine sims
    a  = sign(a)*|a|^3 == a^3 # odd power => plain cube
    echo = a @ r              # [N, 128]

Sharding: data-parallel over X rows across 8 NeuronCores (N_loc=1024/core),
D and r replicated. No collectives.

Host-side layout prep (pure data movement, no math):
    DT     = D.T  (contiguous [128, M])     -> mm1 stationary needs d-major
    r_perm = tile-permuted r so each SBUF partition gets a contiguous 64KB run

Per-core dataflow:
    - DT, r fully resident in SBUF; X normalized+transposed on chip (f32)
    - ss_m = sum_d D^2:  ACT Square(DT chunk)->DTsq,  PE ones-matmul -> psum col
      s = 1/sqrt(ss) per 16-tile group (ACT Sqrt + DVE reciprocal)
    - per m-tile t (128):
        PE  mm1 (f32r): aT[m,n] = DT_tile.T @ XT       (raw-D numerators)
        DVE fused custom op: a3 = (s_m * aT)^3 -> bf16 (single pass from PSUM)
        ACT copy-cast r-tile -> bf16
        PE  mm2 (bf16): echoT[k,n] += r_tile.T @ a3    (PSUM accum over t)
    - epilogue: echoT --PE transpose--> OUT[n,k]
"""

import sys

sys.path.insert(0, "/opt/trn_rl_repo")

import numpy as np

import concourse.bacc as bacc
import concourse.bass as bass
import concourse.tile as tile
from concourse import mybir
from concourse.bass_utils import run_bass_kernel_spmd
from concourse.masks import make_identity
from concourse.bass import ts

# ----------------------------------------------------------------------------
# Custom DVE op: out = (in0 * s0)^3, s0 a per-partition [P,1] scalar.
# One streaming DVE pass (3 ALU stages) replaces ACT-square + DVE-mult.
# ----------------------------------------------------------------------------
from concourse import dve_ops as dvo
from concourse.dve_spec import Spec, Src0, C0, sq, lower, _has_src1
from concourse.dve_uop import DveOpSpec


def _register_cube_op():
    name = "CUBE_SCALED_ANT"
    for op in dvo.OPS:
        if op.name == name:
            return op
    t = Src0 * C0
    spec = Spec(
        body=t * sq(t),
        reference=lambda in0, in1, s0, s1, imm2: (in0.astype(np.float32) * s0) ** 3,
    )
    row = max(dvo._SUB_OPCODE_FOR_NAME.values()) + 1
    assert row < 0x20
    dvo._SUB_OPCODE_FOR_NAME[name] = row
    shas = {}
    for ver in ("v3", "v4"):
        uops = lower(spec, ver=ver)
        shas[ver] = DveOpSpec(
            name=name, opcode=row, uops=uops, rd1_en=_has_src1(spec)
        ).sha(ver)
    op = dvo.DveOp(name, spec, subdim=False, uops_sha=shas)
    dvo.OPS.append(op)
    dvo.CUSTOM_DVE_SPECS[name] = spec
    return op


CUBE_OP = _register_cube_op()

# Problem shapes (hardcoded per contract).
N, M, d = 8192, 16384, 128
NCORES = 8
N_LOC = N // NCORES  # 1024
P = 128
NT = N_LOC // P  # 8 n-tiles per core
MT = M // P  # 128 m-tiles
GRP = 16  # m-tiles per sqrt/recip group
F32 = mybir.dt.float32
F32R = mybir.dt.float32r
BF16 = mybir.dt.bfloat16


def build_kernel(nc: bass.Bass, Xap, DTap, Rap, OUTap, tc: tile.TileContext):
    from contextlib import ExitStack

    with ExitStack() as ctx:
        consts = ctx.enter_context(tc.tile_pool(name="consts", bufs=1))
        big = ctx.enter_context(tc.tile_pool(name="big", bufs=1))
        dsqp = ctx.enter_context(tc.tile_pool(name="dsqp", bufs=2))
        rbfp = ctx.enter_context(tc.tile_pool(name="rbfp", bufs=3))
        a3p = ctx.enter_context(tc.tile_pool(name="a3p", bufs=3))
        scrapp = ctx.enter_context(tc.tile_pool(name="scrapp", bufs=2))
        xnp = ctx.enter_context(tc.tile_pool(name="xnp", bufs=2))
        outp = ctx.enter_context(tc.tile_pool(name="outp", bufs=2))

        pa = ctx.enter_context(tc.tile_pool(name="pa", bufs=2, space="PSUM"))
        pecho = ctx.enter_context(tc.tile_pool(name="pecho", bufs=1, space="PSUM"))
        pss = ctx.enter_context(tc.tile_pool(name="pss", bufs=1, space="PSUM"))
        pt = ctx.enter_context(tc.tile_pool(name="pt", bufs=1, space="PSUM"))

        identb = consts.tile([P, P], BF16)
        make_identity(nc, identb)
        ones = consts.tile([P, 1], BF16)
        nc.vector.memset(ones, 1.0)

        # first ACT op is a Sqrt so walrus loads the sqrt table set (which
        # also contains Copy/Square) once -- avoids a mid-ramp table switch
        tinyf = consts.tile([P, 1], F32)
        nc.vector.memset(tinyf, 1.0)
        nc.scalar.activation(
            out=tinyf, in_=tinyf, func=mybir.ActivationFunctionType.Sqrt
        )

        # ---- bulk loads ------------------------------------------------
        # DT staged in f32 chunks, cast on-chip to bf16 for mm1; sumsq is
        # taken from the rounded (bf16) values so cosines are self-consistent.
        DTbuf = big.tile([P, M], BF16)  # [d, m]
        Rbuf = big.tile([P, MT, d], F32)  # Rbuf[p, t, :] = r[t*128+p, :]
        Rr = Rap.rearrange("(p t) d -> p t d", t=MT)
        CH = M // 16  # 1024 cols / chunk (512 KB)
        CHT = MT // 16  # 8 m-tiles / chunk
        stagep = ctx.enter_context(tc.tile_pool(name="stagep", bufs=3))

        def emit_cast(c):
            stage = stagep.tile([P, CH], F32, tag="dstage")
            nc.sync.dma_start(out=stage, in_=DTap[:, ts(c, CH)])
            nc.scalar.activation(
                out=DTbuf[:, ts(c, CH)],
                in_=stage,
                func=mybir.ActivationFunctionType.Copy,
            )

        # X first (it gates every mm1), contiguous 4KB descriptors via
        # permuted n-tiles: Xbuf[p, i, :] = X[p*NT + i]  (un-permuted at OUT)
        # split in two so the halves ride parallel DMA queues
        Xbuf = big.tile([P, NT, d], F32)
        Xr = Xap.rearrange("(p i) d -> p i d", i=NT)
        nc.sync.dma_start(out=Xbuf[:, : NT // 2, :], in_=Xr[:, : NT // 2, :])
        nc.sync.dma_start(out=Xbuf[:, NT // 2 :, :], in_=Xr[:, NT // 2 :, :])

        # ---- X transpose into XT [d, n_loc], RAW (un-normalized) --------
        # X norm is deferred: echo rows get scaled by sx^3 in the epilogue
        # (cube is homogeneous degree 3), keeping X-prep off the critical path.
        # Copies back from PSUM ride the (ramp-idle) DVE.
        XT = consts.tile([P, N_LOC], BF16)  # [d, n]
        for i in range(NT):
            xb = xnp.tile([P, d], BF16, tag="xb")
            nc.vector.tensor_copy(xb, Xbuf[:, i, :])
            ptb = pt.tile([P, P], BF16, tag="pt")
            nc.tensor.transpose(ptb, xb, identb)
            nc.vector.tensor_copy(XT[:, ts(i, P)], ptb)

        # early D chunks so mm1/ss can start, then r bulk
        emit_cast(0)
        emit_cast(1)
        nc.sync.dma_start(out=Rbuf[:, ts(0, CHT), :], in_=Rr[:, ts(0, CHT), :])
        emit_cast(2)
        emit_cast(3)
        for c in range(1, 16):
            nc.sync.dma_start(
                out=Rbuf[:, ts(c, CHT), :], in_=Rr[:, ts(c, CHT), :]
            )

        # ---- X norms (consumed only by the epilogue; computed in the
        # DVE's idle ramp window) -----------------------------------------
        ssx = consts.tile([P, NT], F32)
        sx3 = consts.tile([P, NT], F32)
        xsq = consts.tile([P, NT, d], F32)
        nc.vector.tensor_mul(
            xsq.rearrange("p a b -> p (a b)"),
            Xbuf.rearrange("p a b -> p (a b)"),
            Xbuf.rearrange("p a b -> p (a b)"),
        )
        nc.vector.tensor_reduce(
            ssx, xsq, axis=mybir.AxisListType.X, op=mybir.AluOpType.add
        )
        nc.scalar.activation(out=sx3, in_=ssx, func=mybir.ActivationFunctionType.Sqrt)
        nc.vector.reciprocal(out=sx3, in_=sx3)
        nc.vector.tensor_mul(ssx, sx3, sx3)  # reuse ssx as sx^2
        nc.vector.tensor_mul(sx3, ssx, sx3)  # sx^3

        # ---- D row norms: ss_m = sum_d DT[:,m]^2 via Square + ones-matmul.
        # Emitted group-by-group, interleaved with the main loop so the
        # pipeline ramps immediately instead of waiting for all of D.
        ss_ps = pss.tile([P, MT], F32)  # psum, col t = ss for m-tile t
        sd = consts.tile([P, MT], F32)

        def emit_ss_group(g):
            # sumsq + rsqrt for m-tiles [g*GRP, (g+1)*GRP)
            for c in range(g * GRP // 8, (g + 1) * GRP // 8):  # 1024-col chunks
                dsq = dsqp.tile([P, 1024], BF16, tag="dsq")
                nc.scalar.activation(
                    out=dsq,
                    in_=DTbuf[:, ts(c, 1024)],
                    func=mybir.ActivationFunctionType.Square,
                )
                for k in range(8):
                    t = 8 * c + k
                    nc.tensor.matmul(
                        ss_ps[:, t : t + 1],
                        lhsT=dsq[:, ts(k, P)],
                        rhs=ones,
                        start=True,
                        stop=True,
                    )
            nc.scalar.activation(
                out=sd[:, ts(g, GRP)],
                in_=ss_ps[:, ts(g, GRP)],
                func=mybir.ActivationFunctionType.Sqrt,
            )
            nc.vector.reciprocal(out=sd[:, ts(g, GRP)], in_=sd[:, ts(g, GRP)])

        emit_ss_group(0)

        # ---- main loop over m-tiles ------------------------------------
        echoT = pecho.tile([P, N_LOC], F32)  # [k, n] psum accumulator
        for t in range(MT):
            aT = pa.tile([P, N_LOC], F32, tag="aT")
            lhsT1 = DTbuf[:, ts(t, P)]
            for c in range(N_LOC // 512):
                nc.tensor.matmul(
                    aT[:, ts(c, 512)],
                    lhsT=lhsT1,
                    rhs=XT[:, ts(c, 512)],
                    start=True,
                    stop=True,
                )

            # fused cube with per-partition norm scale: a3 = (s_m * aT)^3
            a3 = a3p.tile([P, N_LOC], BF16, tag="a3")
            nc.vector._custom_dve(CUBE_OP, out=a3, in0=aT, s0=sd[:, t : t + 1])

            # r tile -> bf16
            rbf = rbfp.tile([P, P], BF16, tag="rbf")
            nc.scalar.activation(
                out=rbf, in_=Rbuf[:, t, :], func=mybir.ActivationFunctionType.Copy
            )

            # mm2: echoT[k, n] += r_tile.T @ a3
            for c in range(N_LOC // 512):
                nc.tensor.matmul(
                    echoT[:, ts(c, 512)],
                    lhsT=rbf,
                    rhs=a3[:, ts(c, 512)],
                    start=(t == 0),
                    stop=(t == MT - 1),
                )

            # prefetch work emitted at tile tails so it sits AFTER this
            # tile's cube in each engine's stream (no head-of-line blocks)
            if t % 8 == 7 and t // 8 + 4 < 16:
                emit_cast(t // 8 + 4)
            if t % GRP == 8 and t // GRP + 1 < MT // GRP:
                emit_ss_group(t // GRP + 1)

        # ---- epilogue: transpose echoT -> OUT [n, k] --------------------
        echoS = consts.tile([P, N_LOC], BF16)
        nc.scalar.activation(
            out=echoS, in_=echoT, func=mybir.ActivationFunctionType.Copy
        )
        OUTr = OUTap.rearrange("(p i) d -> p i d", i=NT)
        for i in range(NT):
            # use the (now idle) aT psum pool for double-buffered transposes
            ptile = pa.tile([P, P], BF16, tag="aT")
            nc.tensor.transpose(ptile, echoS[:, ts(i, P)], identb)
            otile = outp.tile([P, P], F32, tag="otile")
            nc.vector.tensor_scalar_mul(otile, ptile, sx3[:, i : i + 1])
            nc.sync.dma_start(out=OUTr[:, i, :], in_=otile)


_COMPILED = None


def _get_compiled():
    global _COMPILED
    if _COMPILED is None:
        nc = bacc.Bacc(
            "TRN2",
            target_bir_lowering=False,
            debug=False,
            num_devices=1,
        )
        Xap = nc.dram_tensor("X", [N_LOC, d], F32, kind="ExternalInput").ap()
        DTap = nc.dram_tensor("DT", [d, M], F32, kind="ExternalInput").ap()
        Rap = nc.dram_tensor("RP", [M, d], F32, kind="ExternalInput").ap()
        OUTap = nc.dram_tensor("OUT", [N_LOC, d], F32, kind="ExternalOutput").ap()
        with tile.TileContext(nc) as tc:
            build_kernel(nc, Xap, DTap, Rap, OUTap, tc)
        nc.compile()
        _COMPILED = nc
    return _COMPILED


def kernel(X, D, r, _trace=False, _trace_kwargs=None):
    X = np.ascontiguousarray(np.asarray(X), dtype=np.float32)
    D = np.ascontiguousarray(np.asarray(D), dtype=np.float32)
    r = np.ascontiguousarray(np.asarray(r), dtype=np.float32)
    assert X.shape == (N, d) and D.shape == (M, d) and r.shape == (M, d)

    # host-side layout prep (no math): transpose D, tile-permute r
    DT = np.ascontiguousarray(D.T)  # [128, M]
    r_perm = np.ascontiguousarray(
        r.reshape(MT, P, d).transpose(1, 0, 2).reshape(M, d)
    )  # r_perm[p*128+t] = r[t*128+p]

    nc = _get_compiled()
    in_maps = [
        {
            "X": np.ascontiguousarray(X[c * N_LOC : (c + 1) * N_LOC]),
            "DT": DT,
            "RP": r_perm,
        }
        for c in range(NCORES)
    ]
    res = run_bass_kernel_spmd(
        nc,
        in_maps,
        core_ids=list(range(NCORES)),
        trace=_trace,
        **(_trace_kwargs or {}),
    )
    out = np.concatenate([res.results[c]["OUT"] for c in range(NCORES)], axis=0)
    if _trace:
        kernel._last_results = res
    return out


# revision 45
# speedup vs baseline: 1.4632x; 1.0075x over previous
"""Trainium2 Bass kernel for nn_Minerva_37211596652565 (retrieval_knn).

reference:
    Xn = l2norm_rows(X); Dn = l2norm_rows(D)
    a  = Xn @ Dn.T            # [N, M] cosine sims
    a  = sign(a)*|a|^3 == a^3 # odd power => plain cube
    echo = a @ r              # [N, 128]

Sharding: data-parallel over X rows across 8 NeuronCores (N_loc=1024/core),
D and r replicated. No collectives.

Host-side layout prep (pure data movement, no math):
    DT     = D.T  (contiguous [128, M])     -> mm1 stationary needs d-major
    r_perm = tile-permuted r so each SBUF partition gets a contiguous 64KB run

Per-core dataflow:
    - DT, r fully resident in SBUF; X normalized+transposed on chip (f32)
    - ss_m = sum_d D^2:  ACT Square(DT chunk)->DTsq,  PE ones-matmul -> psum col
      s = 1/sqrt(ss) per 16-tile group (ACT Sqrt + DVE reciprocal)
    - per m-tile t (128):
        PE  mm1 (f32r): aT[m,n] = DT_tile.T @ XT       (raw-D numerators)
        DVE fused custom op: a3 = (s_m * aT)^3 -> bf16 (single pass from PSUM)
        ACT copy-cast r-tile -> bf16
        PE  mm2 (bf16): echoT[k,n] += r_tile.T @ a3    (PSUM accum over t)
    - epilogue: echoT --PE transpose--> OUT[n,k]
"""

import sys

sys.path.insert(0, "/opt/trn_rl_repo")

import numpy as np

import concourse.bacc as bacc
import concourse.bass as bass
import concourse.tile as tile
from concourse import mybir
from concourse.bass_utils import run_bass_kernel_spmd
from concourse.masks import make_identity
from concourse.bass import ts

# ----------------------------------------------------------------------------
# Custom DVE op: out = (in0 * s0)^3, s0 a per-partition [P,1] scalar.
# One streaming DVE pass (3 ALU stages) replaces ACT-square + DVE-mult.
# ----------------------------------------------------------------------------
from concourse import dve_ops as dvo
from concourse.dve_spec import Spec, Src0, C0, sq, lower, _has_src1
from concourse.dve_uop import DveOpSpec


def _register_cube_op():
    name = "CUBE_SCALED_ANT"
    for op in dvo.OPS:
        if op.name == name:
            return op
    t = Src0 * C0
    spec = Spec(
        body=t * sq(t),
        reference=lambda in0, in1, s0, s1, imm2: (in0.astype(np.float32) * s0) ** 3,
    )
    row = max(dvo._SUB_OPCODE_FOR_NAME.values()) + 1
    assert row < 0x20
    dvo._SUB_OPCODE_FOR_NAME[name] = row
    shas = {}
    for ver in ("v3", "v4"):
        uops = lower(spec, ver=ver)
        shas[ver] = DveOpSpec(
            name=name, opcode=row, uops=uops, rd1_en=_has_src1(spec)
        ).sha(ver)
    op = dvo.DveOp(name, spec, subdim=False, uops_sha=shas)
    dvo.OPS.append(op)
    dvo.CUSTOM_DVE_SPECS[name] = spec
    return op


CUBE_OP = _register_cube_op()

# Problem shapes (hardcoded per contract).
N, M, d = 8192, 16384, 128
NCORES = 8
N_LOC = N // NCORES  # 1024
P = 128
NT = N_LOC // P  # 8 n-tiles per core
MT = M // P  # 128 m-tiles
GRP = 16  # m-tiles per sqrt/recip group
F32 = mybir.dt.float32
F32R = mybir.dt.float32r
BF16 = mybir.dt.bfloat16


def build_kernel(nc: bass.Bass, Xap, DTap, Rap, OUTap, tc: tile.TileContext):
    from contextlib import ExitStack

    with ExitStack() as ctx:
        consts = ctx.enter_context(tc.tile_pool(name="consts", bufs=1))
        big = ctx.enter_context(tc.tile_pool(name="big", bufs=1))
        dsqp = ctx.enter_context(tc.tile_pool(name="dsqp", bufs=2))
        rbfp = ctx.enter_context(tc.tile_pool(name="rbfp", bufs=3))
        a3p = ctx.enter_context(tc.tile_pool(name="a3p", bufs=3))
        scrapp = ctx.enter_context(tc.tile_pool(name="scrapp", bufs=2))
        xnp = ctx.enter_context(tc.tile_pool(name="xnp", bufs=2))
        outp = ctx.enter_context(tc.tile_pool(name="outp", bufs=4))

        pa = ctx.enter_context(tc.tile_pool(name="pa", bufs=2, space="PSUM"))
        pecho = ctx.enter_context(tc.tile_pool(name="pecho", bufs=1, space="PSUM"))
        pss = ctx.enter_context(tc.tile_pool(name="pss", bufs=1, space="PSUM"))
        pt = ctx.enter_context(tc.tile_pool(name="pt", bufs=1, space="PSUM"))

        identb = consts.tile([P, P], BF16)
        make_identity(nc, identb)
        ones = consts.tile([P, 1], BF16)
        nc.vector.memset(ones, 1.0)

        # first ACT op is a Sqrt so walrus loads the sqrt table set (which
        # also contains Copy/Square) once -- avoids a mid-ramp table switch
        tinyf = consts.tile([P, 1], F32)
        nc.vector.memset(tinyf, 1.0)
        nc.scalar.activation(
            out=tinyf, in_=tinyf, func=mybir.ActivationFunctionType.Sqrt
        )

        # ---- bulk loads ------------------------------------------------
        # DT staged in f32 chunks, cast on-chip to bf16 for mm1; sumsq is
        # taken from the rounded (bf16) values so cosines are self-consistent.
        DTbuf = big.tile([P, M], BF16)  # [d, m]
        Rbuf = big.tile([P, MT, d], F32)  # Rbuf[p, t, :] = r[t*128+p, :]
        Rr = Rap.rearrange("(p t) d -> p t d", t=MT)
        CH = M // 16  # 1024 cols / chunk (512 KB)
        CHT = MT // 16  # 8 m-tiles / chunk
        stagep = ctx.enter_context(tc.tile_pool(name="stagep", bufs=3))

        def emit_cast(c, split=1):
            stage = stagep.tile([P, CH], F32, tag="dstage")
            w = CH // split
            for j in range(split):
                nc.sync.dma_start(
                    out=stage[:, ts(j, w)], in_=DTap[:, c * CH + j * w :][:, :w]
                )
            nc.scalar.activation(
                out=DTbuf[:, ts(c, CH)],
                in_=stage,
                func=mybir.ActivationFunctionType.Copy,
            )

        # X first (it gates every mm1), contiguous 4KB descriptors via
        # permuted n-tiles: Xbuf[p, i, :] = X[p*NT + i]  (un-permuted at OUT)
        # split in two so the halves ride parallel DMA queues
        Xbuf = big.tile([P, NT, d], F32)
        Xr = Xap.rearrange("(p i) d -> p i d", i=NT)
        for j in range(4):
            nc.sync.dma_start(
                out=Xbuf[:, ts(j, NT // 4), :], in_=Xr[:, ts(j, NT // 4), :]
            )

        # ---- X transpose into XT [d, n_loc], RAW (un-normalized) --------
        # X norm is deferred: echo rows get scaled by sx^3 in the epilogue
        # (cube is homogeneous degree 3), keeping X-prep off the critical path.
        # Copies back from PSUM ride the (ramp-idle) DVE.
        XT = consts.tile([P, N_LOC], BF16)  # [d, n]
        for i in range(NT):
            xb = xnp.tile([P, d], BF16, tag="xb")
            nc.vector.tensor_copy(xb, Xbuf[:, i, :])
            ptb = pt.tile([P, P], BF16, tag="pt")
            nc.tensor.transpose(ptb, xb, identb)
            nc.vector.tensor_copy(XT[:, ts(i, P)], ptb)

        # early D chunks so mm1/ss can start, then r bulk.
        # chunk 0 split across 4 DMAs -> 16 engine-pieces -> ~4x faster arrival
        emit_cast(0, split=4)
        emit_cast(1, split=2)
        nc.sync.dma_start(out=Rbuf[:, ts(0, CHT), :], in_=Rr[:, ts(0, CHT), :])
        emit_cast(2)
        emit_cast(3)
        for c in range(1, 16):
            nc.sync.dma_start(
                out=Rbuf[:, ts(c, CHT), :], in_=Rr[:, ts(c, CHT), :]
            )

        ssx = consts.tile([P, NT], F32)
        sx3 = consts.tile([P, NT], F32)
        xsq = consts.tile([P, NT, d], F32)

        # ---- D row norms: ss_m = sum_d DT[:,m]^2 via Square + ones-matmul.
        # Emitted group-by-group, interleaved with the main loop so the
        # pipeline ramps immediately instead of waiting for all of D.
        ss_ps = pss.tile([P, MT], F32)  # psum, col t = ss for m-tile t
        sd = consts.tile([P, MT], F32)

        def emit_ss_group(g):
            # sumsq + rsqrt for m-tiles [g*GRP, (g+1)*GRP)
            for c in range(g * GRP // 8, (g + 1) * GRP // 8):  # 1024-col chunks
                dsq = dsqp.tile([P, 1024], BF16, tag="dsq")
                nc.scalar.activation(
                    out=dsq,
                    in_=DTbuf[:, ts(c, 1024)],
                    func=mybir.ActivationFunctionType.Square,
                )
                for k in range(8):
                    t = 8 * c + k
                    nc.tensor.matmul(
                        ss_ps[:, t : t + 1],
                        lhsT=dsq[:, ts(k, P)],
                        rhs=ones,
                        start=True,
                        stop=True,
                    )
            nc.scalar.activation(
                out=sd[:, ts(g, GRP)],
                in_=ss_ps[:, ts(g, GRP)],
                func=mybir.ActivationFunctionType.Sqrt,
            )
            nc.vector.reciprocal(out=sd[:, ts(g, GRP)], in_=sd[:, ts(g, GRP)])

        emit_ss_group(0)

        # ---- main loop over m-tiles ------------------------------------
        echoT = pecho.tile([P, N_LOC], F32)  # [k, n] psum accumulator
        for t in range(MT):
            if t == 1:
                # X norms (consumed only by the epilogue); emitted after the
                # first cube so the gap-filler can't delay the XT/mm1 chain
                nc.vector.tensor_mul(
                    xsq.rearrange("p a b -> p (a b)"),
                    Xbuf.rearrange("p a b -> p (a b)"),
                    Xbuf.rearrange("p a b -> p (a b)"),
                )
                nc.vector.tensor_reduce(
                    ssx, xsq, axis=mybir.AxisListType.X, op=mybir.AluOpType.add
                )
                nc.scalar.activation(
                    out=sx3, in_=ssx, func=mybir.ActivationFunctionType.Sqrt
                )
                nc.vector.reciprocal(out=sx3, in_=sx3)
                nc.vector.tensor_mul(ssx, sx3, sx3)  # sx^2
                nc.vector.tensor_mul(sx3, ssx, sx3)  # sx^3

            aT = pa.tile([P, N_LOC], F32, tag="aT")
            lhsT1 = DTbuf[:, ts(t, P)]
            for c in range(N_LOC // 512):
                nc.tensor.matmul(
                    aT[:, ts(c, 512)],
                    lhsT=lhsT1,
                    rhs=XT[:, ts(c, 512)],
                    start=True,
                    stop=True,
                )

            # fused cube with per-partition norm scale: a3 = (s_m * aT)^3
            a3 = a3p.tile([P, N_LOC], BF16, tag="a3")
            nc.vector._custom_dve(CUBE_OP, out=a3, in0=aT, s0=sd[:, t : t + 1])

            # r tile -> bf16
            rbf = rbfp.tile([P, P], BF16, tag="rbf")
            nc.scalar.activation(
                out=rbf, in_=Rbuf[:, t, :], func=mybir.ActivationFunctionType.Copy
            )

            # mm2: echoT[k, n] += r_tile.T @ a3
            for c in range(N_LOC // 512):
                nc.tensor.matmul(
                    echoT[:, ts(c, 512)],
                    lhsT=rbf,
                    rhs=a3[:, ts(c, 512)],
                    start=(t == 0),
                    stop=(t == MT - 1),
                )

            # prefetch work emitted at tile tails so it sits AFTER this
            # tile's cube in each engine's stream (no head-of-line blocks)
            if t % 8 == 7 and t // 8 + 4 < 16:
                emit_cast(t // 8 + 4)
            if t % GRP == 8 and t // GRP + 1 < MT // GRP:
                emit_ss_group(t // GRP + 1)

        # ---- epilogue: transpose echoT -> OUT [n, k] --------------------
        # per-tile evac copies + deep out pool so the chain pipelines
        echoS = consts.tile([P, N_LOC], BF16)
        OUTr = OUTap.rearrange("(p i) d -> p i d", i=NT)
        for i in range(NT):
            nc.scalar.activation(
                out=echoS[:, ts(i, P)],
                in_=echoT[:, ts(i, P)],
                func=mybir.ActivationFunctionType.Copy,
            )
            # use the (now idle) aT psum pool for double-buffered transposes
            ptile = pa.tile([P, P], BF16, tag="aT")
            nc.tensor.transpose(ptile, echoS[:, ts(i, P)], identb)
            otile = outp.tile([P, P], F32, tag="otile")
            nc.vector.tensor_scalar_mul(otile, ptile, sx3[:, i : i + 1])
            nc.sync.dma_start(out=OUTr[:, i, :], in_=otile)


_COMPILED = None


def _get_compiled():
    global _COMPILED
    if _COMPILED is None:
        nc = bacc.Bacc(
            "TRN2",
            target_bir_lowering=False,
            debug=False,
            num_devices=1,
        )
        Xap = nc.dram_tensor("X", [N_LOC, d], F32, kind="ExternalInput").ap()
        DTap = nc.dram_tensor("DT", [d, M], F32, kind="ExternalInput").ap()
        Rap = nc.dram_tensor("RP", [M, d], F32, kind="ExternalInput").ap()
        OUTap = nc.dram_tensor("OUT", [N_LOC, d], F32, kind="ExternalOutput").ap()
        with tile.TileContext(nc) as tc:
            build_kernel(nc, Xap, DTap, Rap, OUTap, tc)
        nc.compile()
        _COMPILED = nc
    return _COMPILED


def kernel(X, D, r, _trace=False, _trace_kwargs=None):
    X = np.ascontiguousarray(np.asarray(X), dtype=np.float32)
    D = np.ascontiguousarray(np.asarray(D), dtype=np.float32)
    r = np.ascontiguousarray(np.asarray(r), dtype=np.float32)
    assert X.shape == (N, d) and D.shape == (M, d) and r.shape == (M, d)

    # host-side layout prep (no math): transpose D, tile-permute r
    DT = np.ascontiguousarray(D.T)  # [128, M]
    r_perm = np.ascontiguousarray(
        r.reshape(MT, P, d).transpose(1, 0, 2).reshape(M, d)
    )  # r_perm[p*128+t] = r[t*128+p]

    nc = _get_compiled()
    in_maps = [
        {
            "X": np.ascontiguousarray(X[c * N_LOC : (c + 1) * N_LOC]),
            "DT": DT,
            "RP": r_perm,
        }
        for c in range(NCORES)
    ]
    res = run_bass_kernel_spmd(
        nc,
        in_maps,
        core_ids=list(range(NCORES)),
        trace=_trace,
        **(_trace_kwargs or {}),
    )
    out = np.concatenate([res.results[c]["OUT"] for c in range(NCORES)], axis=0)
    if _trace:
        kernel._last_results = res
    return out
